# revision 23
# baseline (speedup 1.0000x reference)
"""Single-head causal attention (B=8, T=2048, D=1024, HS=64) on 8 trn2
NeuronCores, data-parallel over batch (1 batch element per core).

v4: DMA + PE-density restructure.
  - x^T slab-contiguous (4KB descriptor per partition per 256-col slab)
  - merged t-major QKV projection: stationary = x^T[dc, t-block],
    moving = [Wq|Wk|Wv] (192 cols per load, full 128x128 array) ->
    psum [t, 192]; QK halves transposed (bf16) to Q^T/K^T, V copied
    out t-major directly (no separate V pass, no f32 transposes)
  - scores: row-packed pair of key blocks, one exp per pair; diagonal
    mask multiply moved to gpsimd
  - PV one chunk behind, woven between score pairs; last chunk's PV
    woven into its own pairs (lag 1) to kill the tail
  - out^T ([64 num | 1 denom] x T) DMA'd raw; final divide+transpose on host
"""

import numpy as np
import ml_dtypes

import concourse.bass as bass
import concourse.bacc as bacc
import concourse.tile as tile
from concourse import mybir
from concourse.bass_utils import run_bass_kernel_spmd
from concourse.vector_clock import ScopedClock

B, T, D, HS = 8, 2048, 1024, 64
NCORES = 8
P = 128
ND = D // P        # 8 d-chunks
NB = T // P        # 16 t-blocks
SLAB = 256
NSLAB = T // SLAB
HSLAB = 128
NHSLAB = T // HSLAB
# uneven chunks: small first (exp starts earlier) and last (short tail)
CHUNKS = [(0, 256), (256, 768), (768, 1280), (1280, 1792), (1792, 2048)]

BF16 = mybir.dt.bfloat16
F32 = mybir.dt.float32

# packed bf16 constant block offsets (free-dim)
OFF_W3 = 0           # [128, 8, 192] [Wq|Wk|Wv] chunks
OFF_MASK = 1536      # [128, 512]    [tri | ones | zeros | tri]
OFF_IDB = 2048       # [128, 128]    bf16 identity
CPACK_N = 2176

_MAX_DRAIN_WAITS = 1


def _split_drain_and_barrier(self, tick_clock, wait_clock):
    # Workaround for this walrus build rejecting >1 sem wait on the tail
    # drain: split the waits across a chain of SP nops.
    nc = self.nc
    drain_inst = nc.sync.drain()
    wait_clock.add_sem_waits(
        drain_inst.ins, ScopedClock({None: tick_clock.global_clock})
    )
    si = drain_inst.ins.sync_info
    if si is not None:
        waits = list(si.on_wait)
        if len(waits) > _MAX_DRAIN_WAITS:
            si.on_wait = waits[:_MAX_DRAIN_WAITS]
            drain_inst.ins.sync_info = si
            engines = [nc.tensor, nc.vector, nc.scalar, nc.gpsimd, nc.sync]
            rest = waits[_MAX_DRAIN_WAITS:]
            for k, i in enumerate(range(0, len(rest), _MAX_DRAIN_WAITS)):
                nop = engines[k % len(engines)].nop(nofuse=True)
                nsi = nop.ins.sync_info
                if nsi is None:
                    nsi = mybir.SyncInfo(on_wait=[], on_update=[])
                nsi.on_wait = rest[i : i + _MAX_DRAIN_WAITS]
                nop.ins.sync_info = nsi

    nc.all_engine_barrier()
    assert self.sems is not None
    popped = nc._tile_sem_poison_stack.pop()
    assert popped is self._sem_poison
    nc.clear_and_free_semaphores(list(self.sems.allocated().values()))
    nc.all_engine_barrier()


tile.TileContext._drain_and_barrier = _split_drain_and_barrier


def build_kernel() -> bass.Bass:
    nc = bacc.Bacc("TRN2", target_bir_lowering=False, debug=False, num_devices=NCORES)
    # x^T half-slab-contiguous: xT[p, s, dc, t] = x[s*HSLAB+t, dc*P+p];
    # 2KB descriptor per partition per half-slab, and the first front group
    # gates on just 256KB of x.
    xT = nc.dram_tensor("xT", [P, NHSLAB, ND, HSLAB], BF16, kind="ExternalInput")
    cpack = nc.dram_tensor("cpack", [P, CPACK_N], BF16, kind="ExternalInput")
    # out^T: rows 0:64 = unnormalized numerator, row 64 = denominator
    otT = nc.dram_tensor("otT", [HS + 1, T], F32, kind="ExternalOutput")

    with tile.TileContext(nc) as tc:
        with (
            tc.tile_pool(name="consts", bufs=1) as consts,
            tc.tile_pool(name="xt", bufs=1) as xpool,
            tc.tile_pool(name="qk", bufs=1) as qkpool,
            tc.tile_pool(name="qtm", bufs=2) as qtmpool,
            tc.tile_pool(name="v", bufs=1) as vpool,
            tc.tile_pool(name="e", bufs=22) as epool,
            tc.tile_pool(name="ot", bufs=2) as otpool,
            tc.tile_pool(name="proj_ps", bufs=2, space="PSUM") as ppsum,
            tc.tile_pool(name="tr_ps", bufs=1, space="PSUM") as tpsum,
            tc.tile_pool(name="score_ps", bufs=2, space="PSUM") as spsum,
            tc.tile_pool(name="pv_ps", bufs=1, space="PSUM") as pvpsum,
        ):
            # ---------- all DMAs up front (SP queue) ----------
            cp_sb = consts.tile([P, CPACK_N], BF16)
            # weights first (every front group needs them), then slab 0,
            # then mask+identity, then the rest of x
            nc.sync.dma_start(
                out=cp_sb[:, OFF_W3 : OFF_W3 + 1536],
                in_=cpack[:, OFF_W3 : OFF_W3 + 1536],
            )
            xt_sb = xpool.tile([P, NHSLAB, ND, HSLAB], BF16)
            nc.sync.dma_start(out=xt_sb[:, 0], in_=xT[:, 0])
            nc.sync.dma_start(out=xt_sb[:, 1], in_=xT[:, 1])
            nc.sync.dma_start(
                out=cp_sb[:, OFF_MASK:CPACK_N], in_=cpack[:, OFF_MASK:CPACK_N]
            )
            for s in range(2, NHSLAB):
                nc.sync.dma_start(out=xt_sb[:, s], in_=xT[:, s])

            w3_sb = cp_sb[:, OFF_W3 : OFF_W3 + 1536].rearrange(
                "p (dc m) -> p dc m", m=192
            )
            # [tri | ones | zeros | tri]: one 512-col multiply masks both
            # rows of a diagonal pair
            mask2_sb = cp_sb[:, OFF_MASK : OFF_MASK + 512].rearrange(
                "p (r m) -> p r m", r=2
            )
            idb_sb = cp_sb[:, OFF_IDB : OFF_IDB + P]

            qkA = qkpool.tile([P, T], BF16, tag="qkA")  # Q^T top / K^T bottom
            qkB = qkpool.tile([P, T], BF16, tag="qkB")  # swapped
            v_sb = vpool.tile([P, NB, 72], BF16)
            e_tiles = {}

            # ---------- PE warmup ----------
            # The first slab lands ~3.5us after the preamble; run dummy
            # matmuls meanwhile so the PE p-state is at full clock (and the
            # pipe is hot) when the real front starts.
            # Sized to bridge from preamble end (~7.7us) past slab-0 arrival
            # (~12.2us) with NO gap: an idle PE resets the p-state ramp and
            # the first real fronts would run at 1.2GHz instead of 2.4.
            warm = vpool.tile([P, 512], BF16, tag="warm")
            nc.gpsimd.memset(warm[:], 0.0)
            # denominator ones column for every t-block, one strided memset
            nc.gpsimd.memset(v_sb[:, :, HS : HS + 1], 1.0)
            for wi in range(3):
                wps = ppsum.tile([P, 2, 192], F32, tag="proj", name=f"warm_{wi % 2}")
                nc.tensor.matmul(
                    wps[:], warm[:, 0:P], warm[:, 0:384], start=True, stop=True
                )

            def front_parts(ic):
                """Merged QKV projection for chunk ic in 2-t-block groups,
                as emit-callbacks so score pairs can be woven between."""
                lo, hi = CHUNKS[ic]
                parts = []
                for g0 in range(lo // SLAB, hi // SLAB):

                    def grp(g=g0):
                        tb0 = 2 * g
                        ps3 = ppsum.tile(
                            [P, 2, 192], F32, tag="proj", name=f"p3_{g}"
                        )
                        for blk in range(2):
                            for dc in range(ND):
                                nc.tensor.matmul(
                                    ps3[:, blk, :],
                                    xt_sb[:, 2 * g + blk, dc, :],
                                    w3_sb[:, dc, :],
                                    start=(dc == 0),
                                    stop=(dc == ND - 1),
                                )
                        # QK halves -> bf16 sbuf -> transpose -> qkA/qkB.
                        # qtm cast first: the PE transposes gate on it.
                        qtm = qtmpool.tile(
                            [P, 2, P], BF16, tag="qtm", name=f"qtm_{g}"
                        )
                        nc.vector.tensor_copy(out=qtm[:], in_=ps3[:, :, 0:128])
                        # V out t-major (+ denominator ones column)
                        nc.vector.tensor_copy(
                            out=v_sb[:, tb0 : tb0 + 2, 0:HS],
                            in_=ps3[:, :, 128:192],
                        )
                        psT = tpsum.tile(
                            [P, 2, P], BF16, tag="tr", name=f"tr_{g}"
                        )
                        for blk in range(2):
                            nc.tensor.transpose(
                                psT[:, blk, :], qtm[:, blk, :], idb_sb
                            )
                        gcols = slice(g * SLAB, (g + 1) * SLAB)
                        nc.vector.tensor_copy(out=qkA[:, gcols], in_=psT[:])
                        nc.vector.tensor_copy(
                            out=qkB[0:HS, gcols], in_=psT[HS:P, :, :]
                        )
                        nc.vector.tensor_copy(
                            out=qkB[HS:P, gcols], in_=psT[0:HS, :, :]
                        )

                    parts.append(grp)
                return parts

            def emit_score_pair(ic, g):
                """Row-packed pair (jb0=2g, jb1=2g+1), one merged exp, diag
                mask on gpsimd."""
                clo, chi = CHUNKS[ic]
                W = chi - clo
                jb0, jb1 = 2 * g, 2 * g + 1
                off = max(0, P * jb0 - clo)
                qlo, qhi = clo + off, chi
                psp = spsum.tile([P, 2, SLAB * 2], F32, tag="score", name=f"sps_{ic}_{g}")
                nc.tensor.matmul(
                    psp[:, 0, off:W],
                    qkB[0:HS, jb0 * P : (jb0 + 1) * P],
                    qkA[0:HS, qlo:qhi],
                    start=True,
                    stop=True,
                )
                nc.tensor.matmul(
                    psp[:, 1, off:W],
                    qkA[HS:P, jb1 * P : (jb1 + 1) * P],
                    qkB[HS:P, qlo:qhi],
                    start=True,
                    stop=True,
                )
                et = epool.tile([P, 2, SLAB * 2], BF16, tag="e", name=f"e_{ic}_{g}")
                e_tiles[(ic, g)] = et
                nc.scalar.activation(
                    out=et[:, :, off:W],
                    in_=psp[:, :, off:W],
                    func=mybir.ActivationFunctionType.Exp,
                    scale=float(HS) ** -0.5,
                )
                if P * jb0 >= clo:  # diagonal pair
                    nc.gpsimd.tensor_mul(
                        et[:, :, off : off + 256],
                        et[:, :, off : off + 256],
                        mask2_sb[:],
                    )

            def emit_pv(ic, pv_ps, jb, start=None, stop=None):
                clo, chi = CHUNKS[ic]
                W = chi - clo
                njb = chi // P
                g = jb // 2
                off = max(0, P * (2 * g) - clo)
                nc.tensor.matmul(
                    pv_ps[:, off:W],
                    v_sb[:, jb, 0 : HS + 1],
                    e_tiles[(ic, g)][:, jb & 1, off:W],
                    start=(jb == 0) if start is None else start,
                    stop=(jb == njb - 1) if stop is None else stop,
                )

            def emit_finalize(ic, pv_ps):
                clo, chi = CHUNKS[ic]
                W = chi - clo
                ot = otpool.tile([HS + 1, SLAB * 2], F32, tag="ot", name=f"ot_{ic}")
                nc.vector.tensor_copy(out=ot[:, 0:W], in_=pv_ps[:])
                nc.sync.dma_start(out=otT[:, clo:chi], in_=ot[:, 0:W])

            # ---------- pipeline ----------
            NCK = len(CHUNKS)
            for part in front_parts(0):
                part()
            pv_ps_of = {}
            for ic in range(NCK - 1):
                prev = ic - 1
                pv_jbs = list(range(CHUNKS[prev][1] // P)) if prev >= 0 else []
                if prev >= 0:
                    pv_ps_of[prev] = pvpsum.tile(
                        [HS + 1, CHUNKS[prev][1] - CHUNKS[prev][0]],
                        F32,
                        tag="pv",
                        name=f"pvps_{prev}",
                    )
                tasks = []
                pairs = list(range(CHUNKS[ic][1] // (2 * P)))
                nsteps = len(pairs)
                for si_, g in enumerate(pairs):
                    tasks.append(("pair", g))
                    lo = len(pv_jbs) * si_ // nsteps
                    hi = len(pv_jbs) * (si_ + 1) // nsteps
                    for jb in pv_jbs[lo:hi]:
                        tasks.append(("pv", jb))
                for kind, arg in tasks:
                    if kind == "pair":
                        emit_score_pair(ic, arg)
                    else:
                        emit_pv(prev, pv_ps_of[prev], arg)
                if prev >= 0:
                    emit_finalize(prev, pv_ps_of[prev])
                for part in front_parts(ic + 1):
                    part()

            # Last chunk: finish prev's PV first (single pv psum buffer),
            # then weave this chunk's own PV into its score pairs with a
            # one-pair lag so the tail is just the final pair + finalize.
            ic = NCK - 1
            prev = ic - 1
            pv_ps_of[prev] = pvpsum.tile(
                [HS + 1, CHUNKS[prev][1] - CHUNKS[prev][0]],
                F32,
                tag="pv",
                name=f"pvps_{prev}",
            )
            # Diagonal pair first (longest exp->mask->pv chain), then weave
            # prev's PV through the early pairs; finalize prev (frees the
            # single pv psum buffer); weave this chunk's PV through the late
            # pairs, ending on blocks whose exps are several pairs old.
            prev_jbs = list(range(CHUNKS[prev][1] // P))
            emit_score_pair(ic, 7)
            emit_score_pair(ic, 6)
            for si_, g in enumerate([0, 1, 2, 3]):
                emit_score_pair(ic, g)
                lo = len(prev_jbs) * si_ // 4
                hi = len(prev_jbs) * (si_ + 1) // 4
                for jb in prev_jbs[lo:hi]:
                    emit_pv(prev, pv_ps_of[prev], jb)
            emit_finalize(prev, pv_ps_of[prev])
            pv_ps_of[ic] = pvpsum.tile(
                [HS + 1, CHUNKS[ic][1] - CHUNKS[ic][0]],
                F32,
                tag="pv",
                name=f"pvps_{ic}",
            )
            pv4_seq = [14, 15, 12, 13] + list(range(12))
            emit_score_pair(ic, 4)
            for k, jb in enumerate(pv4_seq[0:8]):
                emit_pv(ic, pv_ps_of[ic], jb, start=(k == 0), stop=False)
            emit_score_pair(ic, 5)
            for k, jb in enumerate(pv4_seq[8:16]):
                emit_pv(ic, pv_ps_of[ic], jb, start=False, stop=(k == 7))
            emit_finalize(ic, pv_ps_of[ic])

    # Hoist ONLY the weights DMA into the entry block ahead of the bass
    # entry barrier: one instruction costs ~0.57us of barrier delay but the
    # first x half-slab then lands ~3us earlier, so the fronts start at
    # ~9us instead of ~11.7us.
    f = nc.main_func
    entry, tile_bb = f.blocks[0], f.blocks[1]
    hoist = [
        i
        for i in tile_bb.instructions
        if isinstance(i, mybir.InstDMACopy)
        and not (i.sync_info and i.sync_info.on_wait)
    ][:1]
    assert len(hoist) == 1
    hoist_ids = {id(i) for i in hoist}
    tile_bb.instructions = [
        i for i in tile_bb.instructions if id(i) not in hoist_ids
    ]
    sp_setup = [
        k
        for k, i in enumerate(entry.instructions)
        if getattr(i, "engine", None) == mybir.EngineType.SP
        and type(i).__name__ in ("InstRegisterMove", "InstTPBBaseLd")
    ]
    idx = max(sp_setup) + 1
    entry.instructions[idx:idx] = hoist

    nc.compile()
    return nc


_NC_CACHE = None


def _get_nc():
    global _NC_CACHE
    if _NC_CACHE is None:
        _NC_CACHE = build_kernel()
    return _NC_CACHE


def _make_in_maps(inputs):
    x, Wq, Wk, Wv = inputs["x"], inputs["Wq"], inputs["Wk"], inputs["Wv"]
    assert x.shape == (B, T, D)
    bf = ml_dtypes.bfloat16

    wqkv = np.concatenate([Wq, Wk, Wv], axis=1)  # [D, 192]
    cpack = np.zeros((P, CPACK_N), dtype=np.float32)
    # w3: cpack[p, dc*192+m] = wqkv[dc*128+p, m]
    cpack[:, OFF_W3 : OFF_W3 + 1536] = (
        wqkv.reshape(ND, P, 192).transpose(1, 0, 2).reshape(P, 1536)
    )
    tri = np.triu(np.ones((P, P), dtype=np.float32))  # keep key <= query
    cpack[:, OFF_MASK : OFF_MASK + P] = tri
    cpack[:, OFF_MASK + P : OFF_MASK + 2 * P] = 1.0
    cpack[:, OFF_MASK + 2 * P : OFF_MASK + 3 * P] = 0.0
    cpack[:, OFF_MASK + 3 * P : OFF_MASK + 4 * P] = tri
    cpack[:, OFF_IDB : OFF_IDB + P] = np.eye(P, dtype=np.float32)
    cpack = cpack.astype(bf)

    in_maps = []
    for b in range(NCORES):
        # [P, NHSLAB, ND, HSLAB]: xTb[p, s, dc, t] = x[b, s*HSLAB+t, dc*P+p]
        xTb = np.ascontiguousarray(
            x[b].reshape(NHSLAB, HSLAB, ND, P).transpose(3, 0, 2, 1)
        ).astype(bf)
        in_maps.append({"xT": xTb, "cpack": cpack})
    return in_maps


def kernel(x, Wq, Wk, Wv):
    in_maps = _make_in_maps({"x": x, "Wq": Wq, "Wk": Wk, "Wv": Wv})
    nc = _get_nc()
    res = run_bass_kernel_spmd(nc, in_maps, list(range(NCORES)))
    outs = []
    for b in range(NCORES):
        ot = res.results[b]["otT"]  # [65, T]
        outs.append((ot[0:HS] / ot[HS : HS + 1]).T)
    return np.ascontiguousarray(np.stack(outs, axis=0)).astype(np.float32)


# revision 24
# speedup vs baseline: 1.0212x; 1.0212x over previous
"""Single-head causal attention (B=8, T=2048, D=1024, HS=64) on 8 trn2
NeuronCores, data-parallel over batch (1 batch element per core).

v4: DMA + PE-density restructure.
  - x^T slab-contiguous (4KB descriptor per partition per 256-col slab)
  - merged t-major QKV projection: stationary = x^T[dc, t-block],
    moving = [Wq|Wk|Wv] (192 cols per load, full 128x128 array) ->
    psum [t, 192]; QK halves transposed (bf16) to Q^T/K^T, V copied
    out t-major directly (no separate V pass, no f32 transposes)
  - scores: row-packed pair of key blocks, one exp per pair; diagonal
    mask multiply moved to gpsimd
  - PV one chunk behind, woven between score pairs; last chunk's PV
    woven into its own pairs (lag 1) to kill the tail
  - out^T ([64 num | 1 denom] x T) DMA'd raw; final divide+transpose on host
"""

import numpy as np
import ml_dtypes

import concourse.bass as bass
import concourse.bacc as bacc
import concourse.tile as tile
from concourse import mybir
from concourse.bass_utils import run_bass_kernel_spmd
from concourse.vector_clock import ScopedClock

B, T, D, HS = 8, 2048, 1024, 64
NCORES = 8
P = 128
ND = D // P        # 8 d-chunks
NB = T // P        # 16 t-blocks
SLAB = 256
NSLAB = T // SLAB
HSLAB = 128
NHSLAB = T // HSLAB
# uneven chunks: small first (exp starts earlier) and last (short tail)
CHUNKS = [(0, 256), (256, 768), (768, 1280), (1280, 1792), (1792, 2048)]

BF16 = mybir.dt.bfloat16
F32 = mybir.dt.float32

# packed bf16 constant block offsets (free-dim)
OFF_W3 = 0           # [128, 8, 192] [Wq|Wk|Wv] chunks
OFF_MASK = 1536      # [128, 512]    [tri | ones | zeros | tri]
OFF_IDB = 2048       # [128, 128]    bf16 identity
CPACK_N = 2176

_MAX_DRAIN_WAITS = 1


def _split_drain_and_barrier(self, tick_clock, wait_clock):
    # Workaround for this walrus build rejecting >1 sem wait on the tail
    # drain: split the waits across a chain of SP nops.
    nc = self.nc
    drain_inst = nc.sync.drain()
    wait_clock.add_sem_waits(
        drain_inst.ins, ScopedClock({None: tick_clock.global_clock})
    )
    si = drain_inst.ins.sync_info
    if si is not None:
        waits = list(si.on_wait)
        if len(waits) > _MAX_DRAIN_WAITS:
            si.on_wait = waits[:_MAX_DRAIN_WAITS]
            drain_inst.ins.sync_info = si
            engines = [nc.tensor, nc.vector, nc.scalar, nc.gpsimd, nc.sync]
            rest = waits[_MAX_DRAIN_WAITS:]
            for k, i in enumerate(range(0, len(rest), _MAX_DRAIN_WAITS)):
                nop = engines[k % len(engines)].nop(nofuse=True)
                nsi = nop.ins.sync_info
                if nsi is None:
                    nsi = mybir.SyncInfo(on_wait=[], on_update=[])
                nsi.on_wait = rest[i : i + _MAX_DRAIN_WAITS]
                nop.ins.sync_info = nsi

    nc.all_engine_barrier()
    assert self.sems is not None
    popped = nc._tile_sem_poison_stack.pop()
    assert popped is self._sem_poison
    nc.clear_and_free_semaphores(list(self.sems.allocated().values()))
    nc.all_engine_barrier()


tile.TileContext._drain_and_barrier = _split_drain_and_barrier


def build_kernel() -> bass.Bass:
    nc = bacc.Bacc("TRN2", target_bir_lowering=False, debug=False, num_devices=NCORES)
    # x^T half-slab-contiguous: xT[p, s, dc, t] = x[s*HSLAB+t, dc*P+p];
    # 2KB descriptor per partition per half-slab, and the first front group
    # gates on just 256KB of x.
    xT = nc.dram_tensor("xT", [P, NHSLAB, ND, HSLAB], BF16, kind="ExternalInput")
    cpack = nc.dram_tensor("cpack", [P, CPACK_N], BF16, kind="ExternalInput")
    # out^T: rows 0:64 = unnormalized numerator, row 64 = denominator
    otT = nc.dram_tensor("otT", [HS + 1, T], F32, kind="ExternalOutput")

    with tile.TileContext(nc) as tc:
        with (
            tc.tile_pool(name="consts", bufs=1) as consts,
            tc.tile_pool(name="xt", bufs=1) as xpool,
            tc.tile_pool(name="qk", bufs=1) as qkpool,
            tc.tile_pool(name="qtm", bufs=2) as qtmpool,
            tc.tile_pool(name="v", bufs=1) as vpool,
            tc.tile_pool(name="e", bufs=22) as epool,
            tc.tile_pool(name="ot", bufs=2) as otpool,
            tc.tile_pool(name="proj_ps", bufs=2, space="PSUM") as ppsum,
            tc.tile_pool(name="tr_ps", bufs=1, space="PSUM") as tpsum,
            tc.tile_pool(name="score_ps", bufs=2, space="PSUM") as spsum,
            tc.tile_pool(name="pv_ps", bufs=1, space="PSUM") as pvpsum,
        ):
            # ---------- all DMAs up front (SP queue) ----------
            cp_sb = consts.tile([P, CPACK_N], BF16)
            # weights first (every front group needs them), then slab 0,
            # then mask+identity, then the rest of x
            nc.sync.dma_start(
                out=cp_sb[:, OFF_W3 : OFF_W3 + 1536],
                in_=cpack[:, OFF_W3 : OFF_W3 + 1536],
            )
            xt_sb = xpool.tile([P, NHSLAB, ND, HSLAB], BF16)
            nc.sync.dma_start(out=xt_sb[:, 0], in_=xT[:, 0])
            nc.sync.dma_start(out=xt_sb[:, 1], in_=xT[:, 1])
            nc.sync.dma_start(
                out=cp_sb[:, OFF_MASK:CPACK_N], in_=cpack[:, OFF_MASK:CPACK_N]
            )
            for s in range(2, NHSLAB):
                nc.sync.dma_start(out=xt_sb[:, s], in_=xT[:, s])

            w3_sb = cp_sb[:, OFF_W3 : OFF_W3 + 1536].rearrange(
                "p (dc m) -> p dc m", m=192
            )
            # [tri | ones | zeros | tri]: one 512-col multiply masks both
            # rows of a diagonal pair
            mask2_sb = cp_sb[:, OFF_MASK : OFF_MASK + 512].rearrange(
                "p (r m) -> p r m", r=2
            )
            idb_sb = cp_sb[:, OFF_IDB : OFF_IDB + P]

            qkA = qkpool.tile([P, T], BF16, tag="qkA")  # Q^T top / K^T bottom
            qkB = qkpool.tile([P, T], BF16, tag="qkB")  # swapped
            v_sb = vpool.tile([P, NB, 72], BF16)
            e_tiles = {}

            # ---------- PE warmup ----------
            # The first slab lands ~3.5us after the preamble; run dummy
            # matmuls meanwhile so the PE p-state is at full clock (and the
            # pipe is hot) when the real front starts.
            # Sized to bridge from preamble end (~7.7us) past slab-0 arrival
            # (~12.2us) with NO gap: an idle PE resets the p-state ramp and
            # the first real fronts would run at 1.2GHz instead of 2.4.
            warm = vpool.tile([P, 512], BF16, tag="warm")
            nc.gpsimd.memset(warm[:], 0.0)
            # denominator ones column for every t-block, one strided memset
            nc.gpsimd.memset(v_sb[:, :, HS : HS + 1], 1.0)
            for wi in range(12):
                wps = ppsum.tile([P, 2, 192], F32, tag="proj", name=f"warm_{wi % 2}")
                nc.tensor.matmul(
                    wps[:], warm[:, 0:P], warm[:, 0:384], start=True, stop=True
                )

            def front_parts(ic):
                """Merged QKV projection for chunk ic in 2-t-block groups,
                as emit-callbacks so score pairs can be woven between."""
                lo, hi = CHUNKS[ic]
                parts = []
                for g0 in range(lo // SLAB, hi // SLAB):

                    def grp(g=g0):
                        tb0 = 2 * g
                        ps3 = ppsum.tile(
                            [P, 2, 192], F32, tag="proj", name=f"p3_{g}"
                        )
                        for blk in range(2):
                            for dc in range(ND):
                                nc.tensor.matmul(
                                    ps3[:, blk, :],
                                    xt_sb[:, 2 * g + blk, dc, :],
                                    w3_sb[:, dc, :],
                                    start=(dc == 0),
                                    stop=(dc == ND - 1),
                                )
                        # QK halves -> bf16 sbuf -> transpose -> qkA/qkB.
                        # qtm cast first: the PE transposes gate on it.
                        qtm = qtmpool.tile(
                            [P, 2, P], BF16, tag="qtm", name=f"qtm_{g}"
                        )
                        nc.vector.tensor_copy(out=qtm[:], in_=ps3[:, :, 0:128])
                        # V out t-major (+ denominator ones column)
                        nc.vector.tensor_copy(
                            out=v_sb[:, tb0 : tb0 + 2, 0:HS],
                            in_=ps3[:, :, 128:192],
                        )
                        psT = tpsum.tile(
                            [P, 2, P], BF16, tag="tr", name=f"tr_{g}"
                        )
                        for blk in range(2):
                            nc.tensor.transpose(
                                psT[:, blk, :], qtm[:, blk, :], idb_sb
                            )
                        gcols = slice(g * SLAB, (g + 1) * SLAB)
                        nc.vector.tensor_copy(out=qkA[:, gcols], in_=psT[:])
                        nc.vector.tensor_copy(
                            out=qkB[0:HS, gcols], in_=psT[HS:P, :, :]
                        )
                        nc.vector.tensor_copy(
                            out=qkB[HS:P, gcols], in_=psT[0:HS, :, :]
                        )

                    parts.append(grp)
                return parts

            def emit_score_pair(ic, g):
                """Row-packed pair (jb0=2g, jb1=2g+1), one merged exp, diag
                mask on gpsimd."""
                clo, chi = CHUNKS[ic]
                W = chi - clo
                jb0, jb1 = 2 * g, 2 * g + 1
                off = max(0, P * jb0 - clo)
                qlo, qhi = clo + off, chi
                psp = spsum.tile([P, 2, SLAB * 2], F32, tag="score", name=f"sps_{ic}_{g}")
                nc.tensor.matmul(
                    psp[:, 0, off:W],
                    qkB[0:HS, jb0 * P : (jb0 + 1) * P],
                    qkA[0:HS, qlo:qhi],
                    start=True,
                    stop=True,
                )
                nc.tensor.matmul(
                    psp[:, 1, off:W],
                    qkA[HS:P, jb1 * P : (jb1 + 1) * P],
                    qkB[HS:P, qlo:qhi],
                    start=True,
                    stop=True,
                )
                et = epool.tile([P, 2, SLAB * 2], BF16, tag="e", name=f"e_{ic}_{g}")
                e_tiles[(ic, g)] = et
                nc.scalar.activation(
                    out=et[:, :, off:W],
                    in_=psp[:, :, off:W],
                    func=mybir.ActivationFunctionType.Exp,
                    scale=float(HS) ** -0.5,
                )
                if P * jb0 >= clo:  # diagonal pair
                    nc.gpsimd.tensor_mul(
                        et[:, :, off : off + 256],
                        et[:, :, off : off + 256],
                        mask2_sb[:],
                    )

            def emit_pv(ic, pv_ps, jb, start=None, stop=None):
                clo, chi = CHUNKS[ic]
                W = chi - clo
                njb = chi // P
                g = jb // 2
                off = max(0, P * (2 * g) - clo)
                nc.tensor.matmul(
                    pv_ps[:, off:W],
                    v_sb[:, jb, 0 : HS + 1],
                    e_tiles[(ic, g)][:, jb & 1, off:W],
                    start=(jb == 0) if start is None else start,
                    stop=(jb == njb - 1) if stop is None else stop,
                )

            def emit_finalize(ic, pv_ps):
                clo, chi = CHUNKS[ic]
                W = chi - clo
                ot = otpool.tile([HS + 1, SLAB * 2], F32, tag="ot", name=f"ot_{ic}")
                nc.vector.tensor_copy(out=ot[:, 0:W], in_=pv_ps[:])
                nc.sync.dma_start(out=otT[:, clo:chi], in_=ot[:, 0:W])

            # ---------- pipeline ----------
            NCK = len(CHUNKS)
            for part in front_parts(0):
                part()
            pv_ps_of = {}
            for ic in range(NCK - 1):
                prev = ic - 1
                pv_jbs = list(range(CHUNKS[prev][1] // P)) if prev >= 0 else []
                if prev >= 0:
                    pv_ps_of[prev] = pvpsum.tile(
                        [HS + 1, CHUNKS[prev][1] - CHUNKS[prev][0]],
                        F32,
                        tag="pv",
                        name=f"pvps_{prev}",
                    )
                tasks = []
                pairs = list(range(CHUNKS[ic][1] // (2 * P)))
                nsteps = len(pairs)
                for si_, g in enumerate(pairs):
                    tasks.append(("pair", g))
                    lo = len(pv_jbs) * si_ // nsteps
                    hi = len(pv_jbs) * (si_ + 1) // nsteps
                    for jb in pv_jbs[lo:hi]:
                        tasks.append(("pv", jb))
                for kind, arg in tasks:
                    if kind == "pair":
                        emit_score_pair(ic, arg)
                    else:
                        emit_pv(prev, pv_ps_of[prev], arg)
                if prev >= 0:
                    emit_finalize(prev, pv_ps_of[prev])
                for part in front_parts(ic + 1):
                    part()

            # Last chunk: finish prev's PV first (single pv psum buffer),
            # then weave this chunk's own PV into its score pairs with a
            # one-pair lag so the tail is just the final pair + finalize.
            ic = NCK - 1
            prev = ic - 1
            pv_ps_of[prev] = pvpsum.tile(
                [HS + 1, CHUNKS[prev][1] - CHUNKS[prev][0]],
                F32,
                tag="pv",
                name=f"pvps_{prev}",
            )
            # Diagonal pair first (longest exp->mask->pv chain), then weave
            # prev's PV through the early pairs; finalize prev (frees the
            # single pv psum buffer); weave this chunk's PV through the late
            # pairs, ending on blocks whose exps are several pairs old.
            prev_jbs = list(range(CHUNKS[prev][1] // P))
            emit_score_pair(ic, 7)
            emit_score_pair(ic, 6)
            for si_, g in enumerate([0, 1, 2, 3]):
                emit_score_pair(ic, g)
                lo = len(prev_jbs) * si_ // 4
                hi = len(prev_jbs) * (si_ + 1) // 4
                for jb in prev_jbs[lo:hi]:
                    emit_pv(prev, pv_ps_of[prev], jb)
            emit_finalize(prev, pv_ps_of[prev])
            pv_ps_of[ic] = pvpsum.tile(
                [HS + 1, CHUNKS[ic][1] - CHUNKS[ic][0]],
                F32,
                tag="pv",
                name=f"pvps_{ic}",
            )
            pv4_seq = [14, 15, 12, 13] + list(range(12))
            emit_score_pair(ic, 4)
            for k, jb in enumerate(pv4_seq[0:8]):
                emit_pv(ic, pv_ps_of[ic], jb, start=(k == 0), stop=False)
            emit_score_pair(ic, 5)
            for k, jb in enumerate(pv4_seq[8:16]):
                emit_pv(ic, pv_ps_of[ic], jb, start=False, stop=(k == 7))
            emit_finalize(ic, pv_ps_of[ic])

    nc.compile()
    return nc


_NC_CACHE = None


def _get_nc():
    global _NC_CACHE
    if _NC_CACHE is None:
        _NC_CACHE = build_kernel()
    return _NC_CACHE


def _make_in_maps(inputs):
    x, Wq, Wk, Wv = inputs["x"], inputs["Wq"], inputs["Wk"], inputs["Wv"]
    assert x.shape == (B, T, D)
    bf = ml_dtypes.bfloat16

    wqkv = np.concatenate([Wq, Wk, Wv], axis=1)  # [D, 192]
    cpack = np.zeros((P, CPACK_N), dtype=np.float32)
    # w3: cpack[p, dc*192+m] = wqkv[dc*128+p, m]
    cpack[:, OFF_W3 : OFF_W3 + 1536] = (
        wqkv.reshape(ND, P, 192).transpose(1, 0, 2).reshape(P, 1536)
    )
    tri = np.triu(np.ones((P, P), dtype=np.float32))  # keep key <= query
    cpack[:, OFF_MASK : OFF_MASK + P] = tri
    cpack[:, OFF_MASK + P : OFF_MASK + 2 * P] = 1.0
    cpack[:, OFF_MASK + 2 * P : OFF_MASK + 3 * P] = 0.0
    cpack[:, OFF_MASK + 3 * P : OFF_MASK + 4 * P] = tri
    cpack[:, OFF_IDB : OFF_IDB + P] = np.eye(P, dtype=np.float32)
    cpack = cpack.astype(bf)

    in_maps = []
    for b in range(NCORES):
        # [P, NHSLAB, ND, HSLAB]: xTb[p, s, dc, t] = x[b, s*HSLAB+t, dc*P+p]
        xTb = np.ascontiguousarray(
            x[b].reshape(NHSLAB, HSLAB, ND, P).transpose(3, 0, 2, 1)
        ).astype(bf)
        in_maps.append({"xT": xTb, "cpack": cpack})
    return in_maps


def kernel(x, Wq, Wk, Wv):
    in_maps = _make_in_maps({"x": x, "Wq": Wq, "Wk": Wk, "Wv": Wv})
    nc = _get_nc()
    res = run_bass_kernel_spmd(nc, in_maps, list(range(NCORES)))
    outs = []
    for b in range(NCORES):
        ot = res.results[b]["otT"]  # [65, T]
        outs.append((ot[0:HS] / ot[HS : HS + 1]).T)
    return np.ascontiguousarray(np.stack(outs, axis=0)).astype(np.float32)
